# revision 24
# baseline (speedup 1.0000x reference)
"""NirvanaDetection kernel for 8 Trainium2 NeuronCores.

Conv(3->32,s1,p1)+BN+ReLU+MaxPool2 -> Conv(32->64,s2,p1)+BN+Sigmoid+MaxPool2
-> per-crop mean score -> threshold.

Strategy (data-parallel over the 512-crop batch, 64 crops per core):
  * maxpool commutes with the monotone per-channel affine+activation
    (BN scale = gamma*rsqrt(var) > 0), so pooling runs directly on raw conv
    accumulator output and BN+activation runs on the 4x smaller pooled tensor.
  * conv1: groups of 4 images. R buffer [36 = (dy,img,c), 128 rows x pitch-130]
    holds the 3 vertically-shifted copies of each plane; rows are padded with
    one zero column on each side so the 3 horizontal taps are plain free-dim
    offsets (reading zeros at the borders). 3 PSUM-accumulating matmuls with a
    block-diagonal [36, 128=(img,co)] weight matrix.
  * conv2: h stored as [(img,c) on partitions, 66x66 zero-padded planes]; both
    the dy and dx taps become free-dim offsets (9 accumulating matmuls,
    K=64=(2 imgs, 32c), M=128=(2 imgs, 64co) block-diagonal).
  * scores: sigmoid activation's accum_out gives per-partition sums; a tiny
    mask matmul reduces across partitions; final scale by 1/16384.
  * detected = scores >= 0.55 computed on host (512 bools).
"""

import numpy as np
from contextlib import ExitStack

import concourse.bass as bass
import concourse.bacc as bacc
import concourse.tile as tile
import concourse.mybir as mybir
from concourse.bass_utils import run_bass_kernel_spmd

F32 = mybir.dt.float32
AF = mybir.ActivationFunctionType
MAX = mybir.AluOpType.max

N_CORES = 8
B, C_IN, HW = 512, 3, 128
C1, C2 = 32, 64
BN_EPS = 1e-5
DIST_THRESHOLD = 0.55
IMGS = B // N_CORES          # 64 images per core
PITCH1 = 130                 # conv1 row pitch (128 + 2 zero pads)
PITCH2 = 66                  # h row pitch (64 + 2 zero pads)


def build_program(n_imgs=IMGS):
    assert n_imgs % 4 == 0
    groups = n_imgs // 4
    nc = bacc.Bacc("TRN2", target_bir_lowering=False, debug=False,
                   num_devices=N_CORES)

    crops_r = nc.dram_tensor("crops_r", [n_imgs // 4, 36, HW, PITCH1], F32,
                             kind="ExternalInput").ap()
    w1bd = nc.dram_tensor("w1bd", [36, 3, 128], F32, kind="ExternalInput").ap()
    w2bd = nc.dram_tensor("w2bd", [128, 9, 128], F32, kind="ExternalInput").ap()
    sb1 = nc.dram_tensor("sb1", [128, 2], F32, kind="ExternalInput").ap()
    sb2 = nc.dram_tensor("sb2", [128, 2], F32, kind="ExternalInput").ap()
    maskd = nc.dram_tensor("maskd", [128, 2], F32, kind="ExternalInput").ap()
    feat_d = nc.dram_tensor("feat", [n_imgs, C2, 16, 16], F32,
                            kind="ExternalOutput").ap()
    scores_d = nc.dram_tensor("scores", [n_imgs], F32,
                              kind="ExternalOutput").ap()

    with tile.TileContext(nc) as tc, ExitStack() as ctx:
        consts = ctx.enter_context(tc.tile_pool(name="consts", bufs=1))
        rpool = ctx.enter_context(tc.tile_pool(name="rbuf", bufs=1))
        hpool = ctx.enter_context(tc.tile_pool(name="hbuf", bufs=1))
        work = ctx.enter_context(tc.tile_pool(name="work", bufs=4))
        ftpool = ctx.enter_context(tc.tile_pool(name="ft", bufs=2))
        ps1 = ctx.enter_context(
            tc.tile_pool(name="ps1", bufs=2, space=bass.MemorySpace.PSUM))
        ps2 = ctx.enter_context(
            tc.tile_pool(name="ps2", bufs=2, space=bass.MemorySpace.PSUM))
        psc = ctx.enter_context(
            tc.tile_pool(name="psc", bufs=1, space=bass.MemorySpace.PSUM))

        w1t = consts.tile([128, 3, 128], F32)
        w2t = consts.tile([128, 9, 128], F32)
        sb1t = consts.tile([128, 2], F32)
        sb2t = consts.tile([128, 2], F32)
        maskt = consts.tile([128, 2], F32)
        nc.sync.dma_start(w1t[0:36], w1bd[:])
        nc.sync.dma_start(w1t[64:100], w1bd[:])
        nc.sync.dma_start(w2t[:], w2bd[:])
        nc.sync.dma_start(sb1t[:], sb1[:])
        nc.sync.dma_start(sb2t[:], sb2[:])
        nc.sync.dma_start(maskt[:], maskd[:])

        # Double-buffered R and h (alternating partition bases 0 / 64).
        R_bufs = [rpool.tile([128, HW, PITCH1], F32, tag=f"R{i}",
                             name=f"Rbuf{i}") for i in range(2)]
        h_bufs = [hpool.tile([128, PITCH2, PITCH2], F32, tag=f"h{i}",
                             name=f"hbuf{i}") for i in range(2)]
        # h pads zeroed once (DVE). R needs no memset — every group's single
        # R-load DMA rewrites the full tile including its zero pads.
        for t in h_bufs:
            nc.vector.memset(t[:], 0.0)

        # Persistent per-pair score accumulator [2, pairs, 2chunks] in PSUM.
        # One extra column is scratch for "observer" matmuls (below).
        pairs = groups * 2
        psct = psc.tile([128, pairs + 1, 2], F32)
        scratch = psct[0:1, pairs, 0:1]

        # Observer matmuls: walrus caps sync-waits per instruction at 1, so
        # PE must observe each producer semaphore via a separate tiny matmul
        # before any real matmul needs two of them at once.
        for ob in (w1t[0:36, 0, 0:1], w2t[0:64, 0, 0:1], maskt[:, 0:1],
                   sb1t[:, 0:1], sb2t[:, 0:1],
                   h_bufs[0][0:36, 0, 0:1], h_bufs[1][0:36, 0, 0:1]):
            nc.tensor.matmul(scratch, ob, ob, start=True, stop=True)

        for g in range(groups):
            Rb = R_bufs[g % 2]
            hb = h_bufs[g % 2]
            p0 = 64 * (g % 2)       # partition base for this group's buffers

            # ---- load R: ONE DMA; host pre-built the padded R layout ----
            nc.gpsimd.dma_start(Rb[p0:p0 + 36, :, :], crops_r[g])
            # PE observes the R-load sem in isolation (1-wait cap).
            nc.tensor.matmul(scratch, Rb[p0:p0 + 36, 0, 0:1],
                             Rb[p0:p0 + 36, 0, 0:1], start=True, stop=True)

            # ---- conv1 + pool + BN/ReLU, 16 chunks of 8 y-rows ----
            for ch in range(16):
                pst = ps1.tile([128, 8, 64, 2], F32)    # (y, x2, xpair)
                for sub in range(2):
                    r0 = 8 * ch + 4 * sub
                    for ddx in range(3):
                        nc.tensor.matmul(
                            pst[:, 4 * sub:4 * sub + 4, :, :],
                            w1t[p0:p0 + 36, ddx, :],
                            Rb[p0:p0 + 36, r0:r0 + 4, ddx:ddx + 128],
                            start=(ddx == 0), stop=(ddx == 2),
                        )
                sp = work.tile([128, 8, 64, 2], F32, tag="sp1")
                # PSUM -> SBUF evacuation (HW allows only one PSUM operand
                # per tensor op); alternate engines to balance load.
                if ch % 2 == 0:
                    nc.scalar.copy(sp[:], pst[:])
                else:
                    nc.vector.tensor_copy(sp[:], pst[:])
                nc.vector.tensor_max(sp[:, :, :, 0], sp[:, :, :, 0],
                                     sp[:, :, :, 1])
                spr = sp[:].rearrange("p (a b) x e -> p a b x e", b=2)
                vp = work.tile([128, 4, 64], F32, tag="vp1")
                nc.vector.tensor_max(vp[:], spr[:, :, 0, :, 0],
                                     spr[:, :, 1, :, 0])
                hsl = hb[:, 1 + 4 * ch:5 + 4 * ch, 1:65]
                nc.scalar.activation(hsl, vp[:], AF.Relu,
                                     bias=sb1t[:, 1:2], scale=sb1t[:, 0:1])

            # ---- conv2 + pool + BN/Sigmoid per image pair ----
            hview = hb[:].rearrange("p (yh yp) (xh xp) -> p yh yp xh xp",
                                    yp=2, xp=2)
            for pp in range(2):
                q0 = 64 * pp
                P = 2 * g + pp
                ft = ftpool.tile([128, 2, 8, 16], F32, tag="ft")
                sums = work.tile([128, 2], F32, tag="sums")
                for cc in range(2):
                    p2t = ps2.tile([128, 16, 16, 2], F32)
                    for k in range(9):
                        dy, dx = k // 3, k % 3
                        rhs = hview[q0:q0 + 64,
                                    (dy >> 1) + 16 * cc:(dy >> 1) + 16 * cc + 16,
                                    dy & 1,
                                    (dx >> 1):(dx >> 1) + 32,
                                    dx & 1]
                        nc.tensor.matmul(
                            p2t[:], w2t[q0:q0 + 64, k, :], rhs,
                            start=(k == 0), stop=(k == 8),
                        )
                    sp2 = work.tile([128, 16, 16, 2], F32, tag="sp2")
                    nc.scalar.copy(sp2[:], p2t[:])
                    nc.vector.tensor_max(sp2[:, :, :, 0], sp2[:, :, :, 0],
                                         sp2[:, :, :, 1])
                    s2r = sp2[:].rearrange("p (a b) x e -> p a b x e", b=2)
                    vp2 = work.tile([128, 8, 16], F32, tag="vp2")
                    nc.vector.tensor_max(vp2[:], s2r[:, :, 0, :, 0],
                                         s2r[:, :, 1, :, 0])
                    fsl = ft[:, cc, :, :]
                    nc.scalar.activation(fsl, vp2[:], AF.Sigmoid,
                                         bias=sb2t[:, 1:2],
                                         scale=sb2t[:, 0:1],
                                         accum_out=sums[:, cc:cc + 1])
                nc.tensor.matmul(psct[0:2, P, :], maskt[:, 0:2],
                                 sums[:, 0:2], start=True, stop=True)
                dst = feat_d[2 * P:2 * P + 2].rearrange(
                    "i co (cc y) x -> i co cc y x", cc=2)
                nc.sync.dma_start(dst, ft[:])

        # ---- final scores ----
        stmp = consts.tile([128, max(pairs, 1), 2], F32)
        ssb = consts.tile([128, max(pairs, 1)], F32)
        nc.scalar.copy(stmp[0:2, :, :], psct[0:2, 0:pairs, :])
        nc.vector.tensor_add(ssb[0:2, :], stmp[0:2, :, 0], stmp[0:2, :, 1])
        nc.scalar.mul(ssb[0:2, :], ssb[0:2, :], 1.0 / 16384.0)
        nc.sync.dma_start(scores_d.rearrange("(p i) -> i p", i=2),
                          ssb[0:2, :])

    nc.compile()
    return nc


def prep_consts(w1, b1, g1, be1, m1, v1, w2, b2, g2, be2, m2, v2):
    w1, w2 = np.asarray(w1, np.float32), np.asarray(w2, np.float32)
    inv1 = (np.asarray(g1) / np.sqrt(np.asarray(v1) + BN_EPS)).astype(np.float32)
    beff1 = ((np.asarray(b1) - np.asarray(m1)) * inv1 + np.asarray(be1)).astype(np.float32)
    inv2 = (np.asarray(g2) / np.sqrt(np.asarray(v2) + BN_EPS)).astype(np.float32)
    beff2 = ((np.asarray(b2) - np.asarray(m2)) * inv2 + np.asarray(be2)).astype(np.float32)

    w1bd = np.zeros((36, 3, 128), np.float32)
    for dy in range(3):
        for img in range(4):
            for c in range(3):
                p = 12 * dy + 3 * img + c
                for ddx in range(3):
                    w1bd[p, ddx, 32 * img:32 * img + 32] = w1[:, c, dy, ddx]

    blk = np.zeros((64, 9, 128), np.float32)
    for i in range(2):
        for c in range(C1):
            for k in range(9):
                dy, dx = k // 3, k % 3
                blk[32 * i + c, k, 64 * i:64 * i + 64] = w2[:, c, dy, dx]
    w2bd = np.concatenate([blk, blk], axis=0)

    co1 = np.arange(128) % 32
    sb1 = np.stack([inv1[co1], beff1[co1]], axis=1).astype(np.float32)
    co2 = np.arange(128) % 64
    sb2 = np.stack([inv2[co2], beff2[co2]], axis=1).astype(np.float32)
    mask = np.zeros((128, 2), np.float32)
    mask[0:64, 0] = 1.0
    mask[64:128, 1] = 1.0
    return dict(w1bd=w1bd, w2bd=w2bd, sb1=sb1, sb2=sb2, maskd=mask)


def prep_crops_r(crops_core):
    """[n,3,128,128] -> [n/4, 36=(dy,img,c), 128, 130] padded R layout."""
    n = crops_core.shape[0]
    g = crops_core.reshape(n // 4, 4, C_IN, HW, HW)
    pad = np.zeros((n // 4, 4, C_IN, HW + 2, HW), np.float32)
    pad[:, :, :, 1:HW + 1, :] = g
    R = np.zeros((n // 4, 36, HW, PITCH1), np.float32)
    for dy in range(3):
        blk = pad[:, :, :, dy:dy + HW, :]
        R[:, 12 * dy:12 * dy + 12, :, 1:HW + 1] = blk.reshape(
            n // 4, 12, HW, HW)
    return R


_PROG_CACHE = {}


def _get_program(n_imgs):
    if n_imgs not in _PROG_CACHE:
        _PROG_CACHE[n_imgs] = build_program(n_imgs)
    return _PROG_CACHE[n_imgs]


def kernel(crops, w1, b1, g1, be1, m1, v1, w2, b2, g2, be2, m2, v2):
    crops = np.asarray(crops, np.float32)
    consts = prep_consts(w1, b1, g1, be1, m1, v1, w2, b2, g2, be2, m2, v2)
    nc = _get_program(IMGS)
    in_maps = []
    for c in range(N_CORES):
        m = {"crops_r": prep_crops_r(crops[c * IMGS:(c + 1) * IMGS])}
        m.update(consts)
        in_maps.append(m)
    res = run_bass_kernel_spmd(nc, in_maps, list(range(N_CORES))).results
    feat = np.concatenate([r["feat"] for r in res], axis=0)
    scores = np.concatenate([r["scores"] for r in res], axis=0)
    detected = scores >= np.float32(DIST_THRESHOLD)
    return feat, scores, detected
